# revision 1
# baseline (speedup 1.0000x reference)
"""BiologicallyInformedAttention TRN2 kernel (8 NeuronCores, axon/PJRT) — v3.

Sharding: B*H = 32 (batch, head-pair) over 8 cores -> core c handles batch
c//2, heads (c%2)*4 .. +4. Weights column-sliced per core; x transposed
host-side so matmuls contract over the partition dim.

v3 architecture (vs baseline): single flat software-pipelined stream.
 - ACT (exp) is the bottleneck engine (~1147ns per [128,1024] tile); a
   custom DVE op (monic factored quartic, squared: ((s+p1)^2+q1)((s+p2)^2+q2)
   all squared ~= k*exp(s/8)) offloads a tunable fraction of exp tiles to the
   otherwise-slack DVE. ACT tiles carry bias=ln(k) so both paths share the
   global scale k, which cancels in softmax.
 - Per kt step: packed score matmuls (2 heads via tile_position row groups),
   exp pair (ACT + sometimes DVE), AV matmuls lagged one step, plus paced
   "extras" (projection/output-projection chunks) to keep PE busy.
 - Projections rotate across phases: proj_qk(ht=1) for repeat r runs during
   r's p=0 attention; proj_qk(ht=0) and proj_v for r+1 run during r's p=1
   attention (v_aug double-buffered by repeat parity; qTr/kTr safe by phase).
 - Prior (+8*pw*eye) applied on PE via bf16 accumulate matmul (pw8 x ident),
   keeping DVE off the scores->exp critical path.
 - Normalize: recip(av row 64 in psum) -> gpsimd broadcast -> DVE mul into
   attnT, staggered over steps; out-proj DMAs direct from PSUM.
Host: out[b] = (outT[2b] + outT[2b+1]).T + (bo + bv @ Wo)   (bv folded: softmax
rows sum to 1, so attn@(v+bv) = attn@v + bv).
"""
import os
import numpy as np
from collections import deque
from contextlib import ExitStack

import concourse.bacc as bacc
import concourse.tile as tile
from concourse import mybir
from concourse.bass_utils import run_bass_kernel_spmd

from concourse.dve_spec import (
    Spec, Src0, C0, C1, C2, C3, _spill_c3_to_src1, lower, _has_src1,
)
from concourse import dve_ops as DO
from concourse.dve_uop import DveOpSpec
from concourse.dve_table_gen import dve_ver_for

B, S, D, H, DH = 4, 2048, 512, 8, 64
HPC = H // 2          # heads per core = 4
W_COLS = HPC * DH     # 256 per-core projection columns
N_CORES = 8

f32 = mybir.dt.float32
f32r = mybir.dt.float32r
bf16 = mybir.dt.bfloat16
f8e4 = mybir.dt.float8e4
AF = mybir.ActivationFunctionType
DR = mybir.MatmulPerfMode.DoubleRow

# exp(0.125*s)*k ~= (((s+P1)^2+Q1)*((s+P2)^2+Q2))^2, fit on s in [-10.2, 10.2]
P1c, Q1c = 28.6807287, 191.7588072
P2c, Q2c = 4.673557346, 1555.248518
LN_K = 28.57072171

EXP_NAME = "EXP_QUARTIC_ANT"


def _exp_quartic_ref(in0, in1, c0, c1, c2):
    x = in0.astype(np.float32)
    a = (x + np.float32(c0 if np.isscalar(c0) else c0)) ** 2 + np.float32(
        c1 if np.isscalar(c1) else c1)
    b = (x + np.float32(c2)) ** 2 + in1.astype(np.float32)
    p = a * b
    return p * p


def _register_exp_op():
    for op in DO.OPS:
        if op.name == EXP_NAME:
            return op
    e = Src0 + C0
    f = e * e
    A = f + C1
    g = Src0 + C2
    h = g * g
    Bq = h + C3
    Pq = A * Bq
    spec = Spec(body=_spill_c3_to_src1(Pq * Pq), reference=_exp_quartic_ref)
    row = max(DO._SUB_OPCODE_FOR_NAME.values()) + 1
    assert row < 0x20, "no free custom-DVE opcode row"
    DO._SUB_OPCODE_FOR_NAME[EXP_NAME] = row
    ver = dve_ver_for("TRN2")
    uops = lower(spec, ver=ver)
    sha = DveOpSpec(name=EXP_NAME, opcode=row, uops=uops,
                    rd1_en=_has_src1(spec)).sha(ver)
    op = DO.DveOp(EXP_NAME, spec, subdim=False, uops_sha={ver: sha})
    DO.OPS.append(op)
    DO.CUSTOM_DVE_SPECS[EXP_NAME] = spec
    return op


EXP_OP = _register_exp_op()

# h1-exp goes to DVE on these kt%16 values (n evenly spread).
_KDVEN = int(os.environ.get("KDVEN", "13"))
DVE_H1 = frozenset(int(round(i * 16 / max(_KDVEN, 1))) % 16 for i in range(_KDVEN))
_ETBUFS = int(os.environ.get("KETBUFS", "8"))
_KPRIOR = os.environ.get("KPRIOR", "pe")
_KEVAC = os.environ.get("KEVAC", "act")   # act: psum evacuations on ScalarE
# fp8 DoubleRow scores: implemented but numerically DEAD for this problem —
# attn-weight quantization error (~1%) lands 1:1 on the output (the output is
# itself a same-scale average), CoreSim-measured 2.3e-2 rel err. Keep off.
_KSC8 = os.environ.get("KSC8", "0") == "1"

_BUILT = {}


def _build(repeat=1):
    nc = bacc.Bacc("TRN2", target_bir_lowering=False)

    xT_d = nc.declare_dram_parameter("xT", [D, S], f32r, isOutput=False)
    wq_d = nc.declare_dram_parameter("wq", [D, W_COLS], f32r, isOutput=False)
    wk_d = nc.declare_dram_parameter("wk", [D, W_COLS], f32r, isOutput=False)
    wv_d = nc.declare_dram_parameter("wv", [D, W_COLS], f32r, isOutput=False)
    wo_d = nc.declare_dram_parameter("wo", [W_COLS, DH], f32r, isOutput=False)
    bq_d = nc.declare_dram_parameter("bq", [W_COLS, 1], f32, isOutput=False)
    bk_d = nc.declare_dram_parameter("bk", [W_COLS, 1], f32, isOutput=False)
    ident_d = nc.declare_dram_parameter("ident", [128, 128], f32, isOutput=False)
    pw8i_d = nc.declare_dram_parameter("pw8i", [128, 128], f32, isOutput=False)
    ones_blk_d = nc.declare_dram_parameter("ones_blk", [128, 8], f32r, isOutput=False)
    outT_d = nc.declare_dram_parameter("outT", [DH, S], f32, isOutput=True)

    with tile.TileContext(nc) as tc, ExitStack() as ctx:
        cp = ctx.enter_context(tc.tile_pool(name="cp", bufs=1))

        # ---------- persistent tiles ----------
        xr = [cp.tile([128, S], f32r, tag=f"xr{i}", name=f"xr{i}") for i in range(4)]
        wqr = [cp.tile([128, W_COLS], f32r, tag=f"wqr{i}", name=f"wqr{i}") for i in range(4)]
        wkr = [cp.tile([128, W_COLS], f32r, tag=f"wkr{i}", name=f"wkr{i}") for i in range(4)]
        wvr = [cp.tile([128, W_COLS], f32r, tag=f"wvr{i}", name=f"wvr{i}") for i in range(4)]
        wor = cp.tile([DH, W_COLS], f32r, tag="wor", name="wor")
        bq_t = cp.tile([128, 2], f32, tag="bq", name="bq")
        bk_t = cp.tile([128, 2], f32, tag="bk", name="bk")
        if _KSC8:
            # fp8 q/k packed for DoubleRow: [64 part, (i=2, s)] with k = 2p+i
            q8t = [cp.tile([64, 2 * S], f8e4, tag=f"q8{p}", name=f"q8{p}")
                   for p in range(2)]
            k8t = [cp.tile([64, 2 * S], f8e4, tag=f"k8{p}", name=f"k8{p}")
                   for p in range(2)]
        else:
            qTr = [cp.tile([128, S], f32r, tag=f"qTr{p}", name=f"qTr{p}")
                   for p in range(2)]
            kTr = [cp.tile([128, S], f32r, tag=f"kTr{p}", name=f"kTr{p}")
                   for p in range(2)]
        nsets = 2 if repeat > 1 else 1
        v_aug = [[cp.tile([128, HPC * 66], f32r, tag=f"va{vs}_{st}", name=f"va{vs}_{st}")
                  for st in range(16)] for vs in range(nsets)]
        attnT = [cp.tile([DH, S], f32r, tag=f"at{h}", name=f"at{h}") for h in range(HPC)]
        outT_s = cp.tile([DH, S], f32, tag="outT", name="outT")
        ident_f = cp.tile([128, 128], f32, tag="idf", name="idf")
        pw8_f = cp.tile([128, 128], f32, tag="pwf", name="pwf")
        ident_b = cp.tile([128, 128], bf16, tag="idb", name="idb")
        pw8_b = cp.tile([128, 128], bf16, tag="pwb", name="pwb")
        q2c = cp.tile([128, 1], f32, tag="q2c", name="q2c")
        lnk = cp.tile([128, 1], f32, tag="lnk", name="lnk")
        ones_blk = cp.tile([128, 8], f32r, tag="ones_blk", name="ones_blk")

        # ---------- loads ----------
        for di in range(4):
            nc.sync.dma_start(wqr[di][:], wq_d[di * 128:(di + 1) * 128, :])
            nc.sync.dma_start(wkr[di][:], wk_d[di * 128:(di + 1) * 128, :])
        for sc4 in range(4):
            s0 = sc4 * 512
            for di in range(4):
                nc.sync.dma_start(xr[di][:, s0:s0 + 512],
                                  xT_d[di * 128:(di + 1) * 128, s0:s0 + 512])
            if sc4 == 1:
                for di in range(4):
                    nc.sync.dma_start(wvr[di][:], wv_d[di * 128:(di + 1) * 128, :])
        for h in range(HPC):
            nc.sync.dma_start(wor[:, h * DH:(h + 1) * DH],
                              wo_d[h * DH:(h + 1) * DH, :])
        for ht in range(2):
            nc.sync.dma_start(bq_t[:, ht:ht + 1], bq_d[ht * 128:(ht + 1) * 128, :])
            nc.sync.dma_start(bk_t[:, ht:ht + 1], bk_d[ht * 128:(ht + 1) * 128, :])
        nc.sync.dma_start(ident_f[:], ident_d[:])
        nc.sync.dma_start(pw8_f[:], pw8i_d[:])
        nc.sync.dma_start(ones_blk[:], ones_blk_d[:])
        nc.vector.tensor_copy(ident_b[:], ident_f[:])
        nc.vector.tensor_copy(pw8_b[:], pw8_f[:])
        nc.vector.memset(q2c[:], Q2c)
        nc.vector.memset(lnk[:], LN_K)
        for vs in range(nsets):
            for st in range(16):
                va = v_aug[vs][st][:].rearrange("p (h c) -> p h c", c=66)
                nc.vector.tensor_copy(
                    va[:, :, DH:66],
                    ones_blk[:].rearrange("p (h c) -> p h c", c=2))

        with tc.tile_pool(name="scp", bufs=2, space="PSUM") as scp, \
             tc.tile_pool(name="avp", bufs=1, space="PSUM") as avp, \
             tc.tile_pool(name="etp", bufs=_ETBUFS) as etp, \
             tc.tile_pool(name="q8p", bufs=4) as q8p, \
             tc.tile_pool(name="nrm", bufs=2) as nrm:

            # ---------- emitters ----------
            def proj_qk_chunk(ht, pair, which):
                # computes a 1024-col pair of s-chunks into one full PSUM slot
                # so the evacuation is a single ACT instruction (amortizes the
                # ~293ns fixed ACT overhead per instruction).
                wr, bias_t = (wqr, bq_t) if which == 0 else (wkr, bk_t)
                s0 = pair * 1024
                pt = scp.tile([128, 1024], f32, tag="sc", name="pj")
                for half in (0, 512):
                    for di in range(4):
                        nc.tensor.matmul(pt[:, half:half + 512],
                                         wr[di][:, ht * 128:(ht + 1) * 128],
                                         xr[di][:, s0 + half:s0 + half + 512],
                                         start=(di == 0), stop=(di == 3))
                if _KSC8:
                    t8 = q8p.tile([128, 1024], f8e4, tag="t8", name="t8")
                    nc.scalar.activation(t8[:], pt[:], AF.Identity,
                                         bias=bias_t[:, ht:ht + 1])
                    dst = q8t if which == 0 else k8t
                    d3 = dst[ht][:].rearrange("p (i s) -> p i s", s=S)
                    nc.sync.dma_start(d3[:, :, s0:s0 + 1024], t8[:])
                elif _KEVAC == "act":
                    dst = qTr if which == 0 else kTr
                    nc.scalar.activation(dst[ht][:, s0:s0 + 1024], pt[:],
                                         AF.Identity, bias=bias_t[:, ht:ht + 1])
                else:
                    dst = qTr if which == 0 else kTr
                    nc.vector.tensor_scalar_add(dst[ht][:, s0:s0 + 1024], pt[:],
                                                bias_t[:, ht:ht + 1])

            def proj_v_chunk(vs, st):
                pv = scp.tile([128, W_COLS], f32, tag="sc", name="pv")
                for di in range(4):
                    nc.tensor.matmul(pv[:], xr[di][:, st * 128:(st + 1) * 128],
                                     wvr[di][:], start=(di == 0), stop=(di == 3))
                va = v_aug[vs][st][:].rearrange("p (h c) -> p h c", c=66)
                if _KEVAC == "act":
                    nc.scalar.activation(va[:, :, 0:DH],
                                         pv[:].rearrange("p (h c) -> p h c", c=DH),
                                         AF.Copy)
                else:
                    nc.vector.tensor_copy(va[:, :, 0:DH],
                                          pv[:].rearrange("p (h c) -> p h c", c=DH))

            def scores(p, qh, kt):
                q0 = qh * 1024
                k0 = kt * 128
                sc0 = scp.tile([128, 1024], f32, tag="sc", name="sc")
                sc1 = scp.tile([128, 1024], f32, tag="sc", name="sc")
                off = k0 - q0
                diag_qc = off // 512 if 0 <= off < 1024 else -1
                if _KPRIOR != "pe":
                    diag_qc = -1
                if _KSC8:
                    q3 = q8t[p][:].rearrange("p (i s) -> p i s", s=S)
                    k3 = k8t[p][:].rearrange("p (i s) -> p i s", s=S)
                for qc in range(2):
                    qq = qc * 512
                    qg = q0 + qq
                    if _KSC8:
                        for sc_t, b32 in ((sc0, 0), (sc1, 32)):
                            nc.tensor.matmul(
                                sc_t[:, qq:qq + 512],
                                k3[b32:b32 + 32, :, k0:k0 + 128],
                                q3[b32:b32 + 32, :, qg:qg + 512],
                                start=True, stop=(qc != diag_qc),
                                tile_position=(b32, 0), perf_mode=DR)
                    else:
                        for sc_t, base in ((sc0, 0), (sc1, 64)):
                            nc.tensor.matmul(
                                sc_t[:, qq:qq + 512],
                                kTr[p][base:base + 64, k0:k0 + 128],
                                qTr[p][base:base + 64, qg:qg + 512],
                                start=True, stop=(qc != diag_qc),
                                tile_position=(base, 0))
                if diag_qc >= 0:
                    # prior += 8*pw*I into the open diag group (closes it)
                    for sc_t in (sc0, sc1):
                        nc.tensor.matmul(sc_t[:, off:off + 128], pw8_b[:],
                                         ident_b[:], start=False, stop=True)
                elif 0 <= off < 1024:
                    for sc_t in (sc0, sc1):
                        nc.vector.tensor_add(sc_t[:, off:off + 128],
                                             sc_t[:, off:off + 128], pw8_f[:])
                return sc0, sc1

            def emit_exps(kt, sc0, sc1):
                et0 = etp.tile([128, 1024], f32r, tag="et", name="et")
                et1 = etp.tile([128, 1024], f32r, tag="et", name="et")
                nc.scalar.activation(et0[:], sc0[:], AF.Exp, scale=0.125,
                                     bias=lnk[:, 0:1])
                if (kt % 16) in DVE_H1:
                    nc.vector._custom_dve(EXP_OP, out=et1[:], in0=sc1[:],
                                          in1=q2c[:], s0=P1c, s1=Q1c, imm2=P2c)
                else:
                    nc.scalar.activation(et1[:], sc1[:], AF.Exp, scale=0.125,
                                         bias=lnk[:, 0:1])
                return et0, et1

            def av_step(vs, p, kt, et0, et1, av0, av1):
                # same-head matmuls adjacent: one LDWEIGHTS per head per kt,
                # and the ACT-produced et0 pair runs while the (often DVE-
                # produced) et1 finishes.
                h0, h1 = 2 * p, 2 * p + 1
                for av, hh, et in ((av0, h0, et0), (av1, h1, et1)):
                    for qc in range(2):
                        qq = qc * 512
                        nc.tensor.matmul(av[:, qq:qq + 512],
                                         v_aug[vs][kt][:, hh * 66:hh * 66 + 66],
                                         et[:, qq:qq + 512],
                                         start=(kt == 0), stop=(kt == 15))

            def norm_recip(av):
                sums = nrm.tile([1, 1024], f32, tag="sums", name="sums")
                nc.vector.tensor_copy(sums[:], av[DH:DH + 1, :])
                recip = nrm.tile([1, 1024], f32, tag="recip", name="recip")
                nc.vector.reciprocal_approx_fast(recip[:], sums[:])
                rB = nrm.tile([DH, 1024], f32, tag="rB", name="rB")
                nc.gpsimd.partition_broadcast(rB[:], recip[:])
                return rB

            def norm_mul(hh, qh, av, rB):
                q0 = qh * 1024
                nc.vector.tensor_mul(attnT[hh][:, q0:q0 + 1024],
                                     av[0:DH, :], rB[:])

            def out_proj(sc4):
                s0 = sc4 * 512
                po = scp.tile([DH, 512], f32, tag="sc", name="po")
                for h in range(HPC):
                    nc.tensor.matmul(po[:], wor[:, h * DH:(h + 1) * DH],
                                     attnT[h][:, s0:s0 + 512],
                                     start=(h == 0), stop=(h == HPC - 1))
                nc.vector.tensor_copy(outT_s[:, s0:s0 + 512], po[:])
                nc.sync.dma_start(outT_d[:, s0:s0 + 512], outT_s[:, s0:s0 + 512])

            # ---------- flat pipelined stream ----------
            pending_av = [None]
            boundary_q = deque()   # per-step closures (normalize/outproj stagger)

            def run_repeat(rep, last):
                vs = rep % 2 if repeat > 1 else 0
                nvs = (rep + 1) % 2 if repeat > 1 else 0
                # extras chunk lists (closures)
                ex_a = deque()   # steps 0..31: this repeat's proj_qk(ht=1)
                for pr in range(2):
                    for w in range(2):
                        ex_a.append(lambda s=pr, w_=w: proj_qk_chunk(1, s, w_))
                ex_b = deque()   # steps 32..62: next repeat's proj_qk(0)+proj_v
                if not last:
                    for pr in range(2):
                        for w in range(2):
                            ex_b.append(lambda s=pr, w_=w: proj_qk_chunk(0, s, w_))
                    for st in range(16):
                        ex_b.append(lambda s=st: proj_v_chunk(nvs, s))

                for p in (0, 1):
                    for qh in (0, 1):
                        av0 = avp.tile([66, 1024], f32, tag="av0", name="av0")
                        av1 = avp.tile([66, 1024], f32, tag="av1", name="av1")
                        for kt in range(16):
                            t = (p * 2 + qh) * 16 + kt
                            sc0, sc1 = scores(p, qh, kt)
                            et0, et1 = emit_exps(kt, sc0, sc1)
                            # boundary steps first: the old qh's final AV +
                            # normalize must be emitted before this qh's AV
                            # writes reuse the av psum slots (WAR ordering).
                            if boundary_q:
                                boundary_q.popleft()()
                            if pending_av[0] is not None:
                                pending_av[0]()
                            pending_av[0] = (
                                lambda v=vs, pp=p, kk=kt, e0=et0, e1=et1,
                                       a0=av0, a1=av1:
                                av_step(v, pp, kk, e0, e1, a0, a1))
                            # pace extras: ex_a on even steps <32, ex_b from 32
                            if t < 32:
                                if ex_a and t % 2 == 0:
                                    ex_a.popleft()()
                            else:
                                if ex_b and (len(ex_b) >= 63 - t):
                                    ex_b.popleft()()
                        # stagger final AV + normalize + out_proj over the
                        # following steps; final AV is folded into step 0 so
                        # it lands before the recips (RAW) and before the next
                        # qh's AV reuses the av slots (WAR).
                        def mk_boundary(pp, qq, a0, a1, av_last):
                            state = {}

                            def s0():
                                if av_last is not None:
                                    av_last()
                                state["r0"] = norm_recip(a0)
                                state["r1"] = norm_recip(a1)
                            def s1():
                                norm_mul(2 * pp, qq, a0, state["r0"])
                            def s2():
                                norm_mul(2 * pp + 1, qq, a1, state["r1"])
                            steps = [s0, s1, s2]
                            if pp == 1:
                                steps.append(lambda: out_proj(2 * qq))
                                steps.append(lambda: out_proj(2 * qq + 1))
                            return steps
                        boundary_q.extend(
                            mk_boundary(p, qh, av0, av1, pending_av[0]))
                        pending_av[0] = None
                # drain remaining extras at end of repeat
                while ex_a:
                    ex_a.popleft()()
                while ex_b:
                    ex_b.popleft()()

            # prologue: repeat 0's proj_qk(ht=0) + proj_v(set 0)
            for pr in range(2):
                for w in range(2):
                    proj_qk_chunk(0, pr, w)
            for st in range(16):
                proj_v_chunk(0, st)

            for rep in range(repeat):
                run_repeat(rep, last=(rep == repeat - 1))
            # epilogue: flush pending AV + boundary chain
            if pending_av[0] is not None:
                pending_av[0]()
                pending_av[0] = None
            while boundary_q:
                boundary_q.popleft()()

    nc.finalize()
    return nc


def _get_nc(repeat=1):
    key = (repeat, _KDVEN, _ETBUFS, _KPRIOR, _KEVAC, _KSC8)
    if key not in _BUILT:
        _BUILT[key] = _build(repeat)
    return _BUILT[key]


def _make_in_maps(x, Wq, bq, Wk, bk, Wv, bv, Wo, bo, prior_weight):
    pw8i = (8.0 * float(prior_weight[0])) * np.eye(128, dtype=np.float32)
    ident = np.eye(128, dtype=np.float32)
    xT = [np.ascontiguousarray(x[b].T) for b in range(B)]
    in_maps = []
    for c in range(N_CORES):
        b, half = c // 2, c % 2
        cs = slice(half * W_COLS, (half + 1) * W_COLS)
        in_maps.append({
            "xT": xT[b],
            "wq": np.ascontiguousarray(Wq[:, cs]),
            "wk": np.ascontiguousarray(Wk[:, cs]),
            "wv": np.ascontiguousarray(Wv[:, cs]),
            "wo": np.ascontiguousarray(Wo[cs, :]),
            "bq": np.ascontiguousarray(bq[cs].reshape(W_COLS, 1)),
            "bk": np.ascontiguousarray(bk[cs].reshape(W_COLS, 1)),
            "ident": ident,
            "pw8i": pw8i,
            "ones_blk": np.ones((128, 8), np.float32),
        })
    return in_maps


def run(inputs, trace=False, trace_cores=None):
    """Execute on 8 cores; returns (output [B,S,DH] f32, BassKernelResults)."""
    args = {k: np.asarray(v) for k, v in inputs.items()}
    nc = _get_nc()
    in_maps = _make_in_maps(
        args["x"], args["Wq"], args["bq"], args["Wk"], args["bk"],
        args["Wv"], args["bv"], args["Wo"], args["bo"], args["prior_weight"])
    res = run_bass_kernel_spmd(
        nc, in_maps, list(range(N_CORES)), trace=trace,
        **({"trace_cores": trace_cores} if trace_cores else {}))
    bo_eff = (args["bo"].astype(np.float64)
              + args["bv"].astype(np.float64) @ args["Wo"].astype(np.float64)
              ).astype(np.float32)
    out = np.empty((B, S, DH), np.float32)
    for b in range(B):
        acc = res.results[2 * b]["outT"] + res.results[2 * b + 1]["outT"]
        out[b] = acc.T + bo_eff
    return out, res


def kernel(**inputs) -> np.ndarray:
    out, _ = run(inputs, trace=False)
    return out

